# revision 3
# baseline (speedup 1.0000x reference)
"""Max-plus (tropical) 2D convolution on 8 TRN2 NeuronCores.

out[b,o,y,x] = max_{c,i,j} ( img[b,c,y+i,x+j] + kernel[o,c,KH-1-i,KW-1-j] )

Sharding: core = b*2 + g  (b in 0..3 data-parallel over batch,
g in 0..1 tensor-parallel over halves of C_OUT). No cross-core comm.

Per-core compute: host-side im2col gives patches T[p, r] with p = y*WO+x
(pixels, on partitions) and r = (c,i,j) (reduction, on free axis, R=200).
One fused DVE tensor_tensor_reduce per (o, 128-pixel block):
    accum[p] = max_r ( T[p, r] + w[o, r] )
"""

import sys

import numpy as np

if "/opt/trn_rl_repo" not in sys.path:
    sys.path.insert(0, "/opt/trn_rl_repo")

B, C_IN, H, W = 4, 8, 128, 128
C_OUT, KH, KW = 16, 5, 5
HO, WO = H - KH + 1, W - KW + 1  # 124, 124
P = HO * WO  # 15376 output pixels per (b, o)
R = C_IN * KH * KW  # 200 reduction terms
NBLK = (P + 127) // 128  # 121 pixel blocks
PPAD = NBLK * 128  # 15488
OG = 2  # groups of output channels
O_LOC = C_OUT // OG  # 8 output channels per core
N_CORES = 8


def _get_ttmr_op():
    """Register (once) a custom DVE op: accum_out = max(s0, max_k(in0[k]+in1[k]))."""
    from concourse import dve_ops as DO
    from concourse.dve_spec import C0, Spec, Src0, Src1, _has_src1, lower, maxx
    from concourse.dve_uop import DveOpSpec

    name = "TT_MAX_REDUCE_ANT"
    for op in DO.OPS:
        if op.name == name:
            return op
    spec = Spec(body=Src0 + Src1, accum=maxx, accum_init=C0)
    row = max(DO._SUB_OPCODE_FOR_NAME.values()) + 1
    assert row < 0x20, "custom-DVE row field overflow"
    DO._SUB_OPCODE_FOR_NAME[name] = row
    shas = {}
    for ver in ("v3", "v4"):
        s = DveOpSpec(
            name=name, opcode=row, uops=lower(spec, ver=ver), rd1_en=_has_src1(spec)
        )
        shas[ver] = s.sha(ver)
    op = DO.DveOp(name, spec, subdim=False, uops_sha=shas)
    DO.OPS.append(op)
    DO.CUSTOM_DVE_SPECS[name] = spec
    return op


def _build_program():
    import concourse.bacc as bacc
    import concourse.mybir as mybir
    from concourse.tile import TileContext

    ttmr = _get_ttmr_op()
    f32 = mybir.dt.float32
    nc = bacc.Bacc("TRN2", target_bir_lowering=False, debug=False)

    t_dram = nc.dram_tensor("t", [NBLK, 128, R], f32, kind="ExternalInput")
    wb_dram = nc.dram_tensor("wb", [128, O_LOC * R], f32, kind="ExternalInput")
    out_dram = nc.dram_tensor("out", [NBLK, 128, O_LOC], f32, kind="ExternalOutput")

    with TileContext(nc) as tc:
        with (
            tc.tile_pool(name="wbp", bufs=1) as wbp,
            tc.tile_pool(name="tin", bufs=4) as tinp,
            tc.tile_pool(name="op", bufs=4) as outp,
            tc.tile_pool(name="dm", bufs=2) as dmp,
        ):
            wb = wbp.tile([128, O_LOC * R], f32)
            nc.sync.dma_start(out=wb[:, :], in_=wb_dram[:, :])
            for blk in range(NBLK):
                tin = tinp.tile([128, R], f32)
                nc.sync.dma_start(out=tin[:, :], in_=t_dram[blk, :, :])
                ot = outp.tile([128, O_LOC], f32)
                dummy = dmp.tile([128, 1], f32)
                for o in range(O_LOC):
                    nc.vector._custom_dve(
                        ttmr,
                        out=dummy.broadcast_to((128, R)),
                        in0=tin[:, :],
                        in1=wb[:, o * R : (o + 1) * R],
                        s0=-1.0e30,
                        s1=0.0,
                        imm2=0.0,
                        accum_out=ot[:, o : o + 1],
                    )
                nc.sync.dma_start(out=out_dram[blk, :, :], in_=ot[:, :])
    nc.finalize()
    return nc


def _host_shards(img: np.ndarray, kern: np.ndarray):
    """im2col on host: per-batch patches + per-group broadcast weights."""
    from numpy.lib.stride_tricks import sliding_window_view

    kflip = kern[:, :, ::-1, ::-1]
    wmat = np.ascontiguousarray(kflip.reshape(C_OUT, R))  # [16, 200], r=(c,i,j)

    sw = sliding_window_view(img, (KH, KW), axis=(2, 3))  # [B,C,HO,WO,KH,KW]
    t_full = sw.transpose(0, 2, 3, 1, 4, 5).reshape(B, P, R)  # [B, p=(y,x), r=(c,i,j)]
    t_pad = np.zeros((B, PPAD, R), np.float32)
    t_pad[:, :P] = t_full

    in_maps = []
    for core in range(N_CORES):
        b, g = divmod(core, OG)
        wb = np.tile(
            wmat[g * O_LOC : (g + 1) * O_LOC].reshape(1, O_LOC * R), (128, 1)
        ).astype(np.float32)
        in_maps.append(
            {
                "t": t_pad[b].reshape(NBLK, 128, R),
                "wb": wb,
            }
        )
    return in_maps


def _run(in_maps, trace=False, **kwargs):
    from concourse.bass_utils import run_bass_kernel_spmd

    nc = _build_program()
    return run_bass_kernel_spmd(
        nc, in_maps, core_ids=list(range(N_CORES)), trace=trace, **kwargs
    )


def kernel(**inputs) -> np.ndarray:
    img = np.ascontiguousarray(np.asarray(inputs["img"], dtype=np.float32))
    kern = np.ascontiguousarray(np.asarray(inputs["kernel"], dtype=np.float32))

    in_maps = _host_shards(img, kern)
    res = _run(in_maps)

    out = np.empty((B, C_OUT, HO, WO), np.float32)
    for core in range(N_CORES):
        b, g = divmod(core, OG)
        o_core = res.results[core]["out"].reshape(PPAD, O_LOC)[:P]  # [15376, 8]
        out[b, g * O_LOC : (g + 1) * O_LOC] = np.ascontiguousarray(o_core.T).reshape(
            O_LOC, HO, WO
        )
    return out


# revision 4
# speedup vs baseline: 1.5265x; 1.5265x over previous
"""Max-plus (tropical) 2D convolution on 8 TRN2 NeuronCores.

out[b,o,y,x] = max_{c,i,j} ( img[b,c,y+i,x+j] + kernel[o,c,KH-1-i,KW-1-j] )

Sharding: core = b*2 + g  (b in 0..3 data-parallel over batch,
g in 0..1 tensor-parallel over halves of C_OUT). No cross-core comm.

Per-core compute: host-side im2col gives patches T[p, r] with p = y*WO+x
(pixels, on partitions) and r = (c,i,j) (reduction, on free axis, R=200).
One fused DVE instruction per (o, 128-pixel block):
    accum[p] = max(init, max_r ( T[p, r] + w[o, r] ))
using a hand-authored custom DVE op (TTMR_FLUSH2) that runs in 2x_1port
perf mode on fp16 streams (2 elems/lane/cycle) and flushes the
accumulator to the dst stream in a third uop state (no separate
READ_ACCUMULATOR instruction).
"""

import sys

import numpy as np

if "/opt/trn_rl_repo" not in sys.path:
    sys.path.insert(0, "/opt/trn_rl_repo")

B, C_IN, H, W = 4, 8, 128, 128
C_OUT, KH, KW = 16, 5, 5
HO, WO = H - KH + 1, W - KW + 1  # 124, 124
P = HO * WO  # 15376 output pixels per (b, o)
R = C_IN * KH * KW  # 200 reduction terms
NBLK = (P + 127) // 128  # 121 pixel blocks
PPAD = NBLK * 128  # 15488
OG = 2  # groups of output channels
O_LOC = C_OUT // OG  # 8 output channels per core
N_CORES = 8

OP_NAME = "TTMR_FLUSH2"
ACC_INIT = -60000.0  # > -fp16_max; every real term beats it


# --------------------------------------------------------------------------
# Custom DVE op: 3-state program (seed / steady / flush), 1x + 2x variants.
# --------------------------------------------------------------------------
def _build_uops():
    from concourse.dve_uop import (
        AluInp,
        AluOp,
        DelayInp,
        InpSel,
        OutPath,
        OutSel,
        Trigger,
        UopConfig,
        UopDpConfig,
    )

    inp = [
        InpSel.ZERO,
        InpSel.SRC_0,  # -> PREV_DELAY_0 at stage 0
        InpSel.SRC_1,  # -> PREV_DELAY_1
        InpSel.CONST_0,  # -> PREV_DELAY_2
        InpSel.SRC_0_HI,  # -> PREV_DELAY_3 (2x mode)
        InpSel.SRC_1_HI,  # -> PREV_DELAY_4 (2x mode)
        InpSel.ZERO,
        InpSel.ZERO,
    ]
    inp_en = [0, 1, 1, 1, 1, 1, 0, 0]

    def base(kind):
        u = UopConfig()
        u.inp = list(inp)
        u.inp_enable = list(inp_en)
        u.accum_enabled = 1
        if kind == "seed":
            u.require_inp0 = 0
            u.require_inp1 = 0
            u.repeat_count = 1
            u.trigger = (Trigger.COUNT, Trigger.NONE, Trigger.NONE)
            u.next_uop = (1, 0, 0)
        elif kind == "steady":
            u.require_inp0 = 1
            u.require_inp1 = 1
            u.trigger = (Trigger.SRC_TENSOR_DONE, Trigger.NONE, Trigger.NONE)
            u.next_uop = (2, 0, 0)
        else:  # flush
            u.require_inp0 = 0
            u.require_inp1 = 0
            u.repeat_count = 1
            u.trigger = (Trigger.COUNT, Trigger.NONE, Trigger.NONE)
            u.next_uop = (0, 0, 0)
        return u

    def byp(a_inp=AluInp.PREV_ALU_OUT, lanes=(0, 1)):
        d = UopDpConfig().enable_alu(AluOp.BYPASS, a_inp)
        d.pass_through_delay(*lanes)
        return d

    # ---- 1x variant: accumulate at stage 1 ----
    def seed_1x():
        u = base("seed")
        d0 = UopDpConfig().enable_alu(
            AluOp.ADD, AluInp.PREV_DELAY_0, AluInp.PREV_DELAY_1
        )
        d0.pass_through_delay(0, 1, 2)
        d1 = byp(AluInp.PREV_DELAY_2, lanes=(0, 1, 2))  # CONST_0 -> stage1 flop
        u.datapath_config = [d0, d1] + [byp() for _ in range(6)]
        return u

    def steady_1x():
        u = base("steady")
        d0 = UopDpConfig().enable_alu(
            AluOp.ADD, AluInp.PREV_DELAY_0, AluInp.PREV_DELAY_1
        )
        d0.pass_through_delay(0, 1, 2)
        d1 = UopDpConfig().enable_alu(
            AluOp.MAX, AluInp.CURR_ALU_OUT, AluInp.PREV_ALU_OUT
        )
        d1.enable_delay_from_src(DelayInp.PREV_ALU_OUT, 0).pass_through_delay(1, 2)
        u.datapath_config = [d0, d1] + [byp() for _ in range(6)]
        return u

    def flush_1x():
        u = base("flush")
        d0 = UopDpConfig().enable_alu(AluOp.BYPASS, AluInp.PREV_DELAY_0)
        d1 = UopDpConfig().enable_alu(AluOp.BYPASS, AluInp.CURR_ALU_OUT)
        u.datapath_config = [d0, d1] + [byp() for _ in range(6)]
        u.enable_output(OutSel.ALU_OUT, OutPath.WR0_LO)
        return u

    # ---- 2x variant: lo add @0, hi add @1, pair-max @2, accumulate @3 ----
    def seed_2x():
        u = base("seed")
        d0 = UopDpConfig().enable_alu(
            AluOp.ADD, AluInp.PREV_DELAY_0, AluInp.PREV_DELAY_1
        )
        d0.pass_through_delay(0, 1, 2, 3, 4)
        d1 = byp(AluInp.PREV_DELAY_2, lanes=(1, 2, 3, 4))  # CONST_0 onto ALU path
        d1.enable_delay_from_src(DelayInp.PREV_ALU_OUT, 0)
        d2 = byp()
        d3 = byp()  # stage3 flop <- CONST_0
        u.datapath_config = [d0, d1, d2, d3] + [byp() for _ in range(4)]
        return u

    def steady_2x():
        u = base("steady")
        d0 = UopDpConfig().enable_alu(
            AluOp.ADD, AluInp.PREV_DELAY_0, AluInp.PREV_DELAY_1
        )
        d0.pass_through_delay(0, 1, 2, 3, 4)
        d1 = UopDpConfig().enable_alu(
            AluOp.ADD, AluInp.PREV_DELAY_3, AluInp.PREV_DELAY_4
        )
        d1.enable_delay_from_src(DelayInp.PREV_ALU_OUT, 0).pass_through_delay(
            1, 2, 3, 4
        )
        d2 = UopDpConfig().enable_alu(
            AluOp.MAX, AluInp.PREV_ALU_OUT, AluInp.PREV_DELAY_0
        )
        d2.enable_delay_from_src(DelayInp.PREV_ALU_OUT, 1).pass_through_delay(0)
        d3 = UopDpConfig().enable_alu(
            AluOp.MAX, AluInp.CURR_ALU_OUT, AluInp.PREV_ALU_OUT
        )
        d3.pass_through_delay(0, 1)
        u.datapath_config = [d0, d1, d2, d3] + [byp() for _ in range(4)]
        return u

    def flush_2x():
        u = base("flush")
        d0 = UopDpConfig().enable_alu(AluOp.BYPASS, AluInp.PREV_DELAY_0)
        d1 = byp()
        d2 = byp()
        d3 = UopDpConfig().enable_alu(AluOp.BYPASS, AluInp.CURR_ALU_OUT)
        d3.pass_through_delay(0, 1)
        u.datapath_config = [d0, d1, d2, d3] + [byp() for _ in range(4)]
        u.enable_output(OutSel.ALU_OUT, OutPath.WR0_LO)
        u.enable_output(OutSel.ALU_OUT, OutPath.WR0_HI)
        return u

    return [seed_1x(), steady_1x(), flush_1x()], [seed_2x(), steady_2x(), flush_2x()]


_COMPILED: dict = {}


def _compile_spec(ver):
    if ver not in _COMPILED:
        import concourse.dve_ops as DO
        from concourse.dve_uop import DveOpSpec

        row = DO._SUB_OPCODE_FOR_NAME[OP_NAME]
        uops_1x, uops_2x = _build_uops()
        s = DveOpSpec(
            name=OP_NAME,
            opcode=row,
            uops=uops_1x,
            rd1_en=True,
            uops_2x=uops_2x,
            perf_max=1,
        )
        s.validate(ver)
        _COMPILED[ver] = s
    return _COMPILED[ver]


def _register_op():
    import concourse.dve_ops as DO
    from concourse.dve_spec import C0, Spec, Src0, Src1, maxx

    for op in DO.OPS:
        if op.name == OP_NAME:
            return op
    spec = Spec(body=Src0 + Src1, accum=maxx, accum_init=C0)
    row = max(DO._SUB_OPCODE_FOR_NAME.values()) + 1
    assert row < 0x20, "custom-DVE row field overflow"
    DO._SUB_OPCODE_FOR_NAME[OP_NAME] = row
    shas = {ver: _compile_spec(ver).sha(ver) for ver in ("v3", "v4")}

    class DveOp2x(DO.DveOp):
        def compile(self, ver):
            return _compile_spec(ver)

    op = DveOp2x(OP_NAME, spec, subdim=False, uops_sha=shas)
    DO.OPS.append(op)
    DO.CUSTOM_DVE_SPECS[OP_NAME] = spec
    return op


def _emit(nc, op, *, out, in0, in1, s0):
    """Emit one fused max-plus-reduce instruction (perf_max=1 -> 2x on fp16)."""
    import concourse.bass_isa as bass_isa
    import concourse.mybir as mybir
    from concourse.dve_ops import get_dve_sub_opcode
    from concourse.dve_table_gen import dve_ver_for

    vec = nc.vector
    if op.name not in nc.m.ant_custom_dve_ops:
        nc.m.ant_custom_dve_ops = sorted({*nc.m.ant_custom_dve_ops, op.name})
    op.compile(dve_ver_for(nc.trn_type))
    shape = bass_isa.CustomDveShape.TTSS
    isa_opcode = nc.isa.Opcode[
        f"NEURON_ISA_TPB_OPCODE_CUSTOM_DVE_ANT_{shape.slot()}"
    ].value
    ins = [
        vec.lower_ap(in0, for_isa=True, opt=True),
        vec.lower_ap(in1, for_isa=True, opt=True),
        mybir.ImmediateValue(dtype=mybir.dt.float32, value=float(s0)),
        mybir.ImmediateValue(dtype=mybir.dt.float32, value=0.0),
    ]
    outs = [vec.lower_ap(out, for_isa=True, opt=True)]
    return vec.add_instruction(
        bass_isa.InstCustomDveAnt(
            name=nc.get_next_instruction_name(),
            op_name=op.name,
            rd1_en=True,
            subdim=0,
            imm2=0.0,
            shape=shape,
            row=get_dve_sub_opcode(op.name),
            isa_opcode=isa_opcode,
            ins=ins,
            outs=outs,
            perf_max=1,
        )
    )


# --------------------------------------------------------------------------
# Program: per core, NBLK x O_LOC fused reduce instructions.
# --------------------------------------------------------------------------
def _build_program():
    import concourse.bacc as bacc
    import concourse.mybir as mybir
    from concourse.tile import TileContext

    ttmr = _register_op()
    f16 = mybir.dt.float16
    nc = bacc.Bacc("TRN2", target_bir_lowering=False, debug=False)

    t_dram = nc.dram_tensor("t", [NBLK, 128, R], f16, kind="ExternalInput")
    wb_dram = nc.dram_tensor("wb", [128, O_LOC * R], f16, kind="ExternalInput")
    out_dram = nc.dram_tensor(
        "out", [NBLK, 128, 2 * O_LOC], f16, kind="ExternalOutput"
    )

    with TileContext(nc) as tc:
        with (
            tc.tile_pool(name="wbp", bufs=1) as wbp,
            tc.tile_pool(name="tin", bufs=4) as tinp,
            tc.tile_pool(name="op", bufs=4) as outp,
        ):
            wb = wbp.tile([128, O_LOC * R], f16)
            nc.sync.dma_start(out=wb[:, :], in_=wb_dram[:, :])
            for blk in range(NBLK):
                tin = tinp.tile([128, R], f16)
                nc.sync.dma_start(out=tin[:, :], in_=t_dram[blk, :, :])
                ot = outp.tile([128, 2 * O_LOC], f16)
                for o in range(O_LOC):
                    _emit(
                        nc,
                        ttmr,
                        out=ot[:, 2 * o : 2 * o + 2],
                        in0=tin[:, :],
                        in1=wb[:, o * R : (o + 1) * R],
                        s0=ACC_INIT,
                    )
                nc.sync.dma_start(out=out_dram[blk, :, :], in_=ot[:, :])
    nc.finalize()
    return nc


def _host_shards(img: np.ndarray, kern: np.ndarray):
    """im2col on host (fp16): per-batch patches + per-group broadcast weights."""
    from numpy.lib.stride_tricks import sliding_window_view

    kflip = kern[:, :, ::-1, ::-1]
    wmat = np.ascontiguousarray(kflip.reshape(C_OUT, R))  # [16, 200], r=(c,i,j)

    sw = sliding_window_view(img, (KH, KW), axis=(2, 3))  # [B,C,HO,WO,KH,KW]
    t_full = sw.transpose(0, 2, 3, 1, 4, 5).reshape(B, P, R)
    t_pad = np.zeros((B, PPAD, R), np.float16)
    t_pad[:, :P] = t_full.astype(np.float16)

    in_maps = []
    for core in range(N_CORES):
        b, g = divmod(core, OG)
        wb = np.tile(
            wmat[g * O_LOC : (g + 1) * O_LOC].reshape(1, O_LOC * R), (128, 1)
        ).astype(np.float16)
        in_maps.append(
            {
                "t": t_pad[b].reshape(NBLK, 128, R),
                "wb": wb,
            }
        )
    return in_maps


def _run(in_maps, trace=False, **kwargs):
    from concourse.bass_utils import run_bass_kernel_spmd

    nc = _build_program()
    return run_bass_kernel_spmd(
        nc, in_maps, core_ids=list(range(N_CORES)), trace=trace, **kwargs
    )


def kernel(**inputs) -> np.ndarray:
    img = np.ascontiguousarray(np.asarray(inputs["img"], dtype=np.float32))
    kern = np.ascontiguousarray(np.asarray(inputs["kernel"], dtype=np.float32))

    in_maps = _host_shards(img, kern)
    res = _run(in_maps)

    out = np.empty((B, C_OUT, HO, WO), np.float32)
    for core in range(N_CORES):
        b, g = divmod(core, OG)
        o_core = (
            res.results[core]["out"]
            .reshape(PPAD, 2 * O_LOC)[:P, ::2]
            .astype(np.float32)
        )  # [15376, 8]
        out[b, g * O_LOC : (g + 1) * O_LOC] = np.ascontiguousarray(o_core.T).reshape(
            O_LOC, HO, WO
        )
    return out


# revision 5
# speedup vs baseline: 1.6666x; 1.0918x over previous
"""Max-plus (tropical) 2D convolution on 8 TRN2 NeuronCores.

out[b,o,y,x] = max_{c,i,j} ( img[b,c,y+i,x+j] + kernel[o,c,KH-1-i,KW-1-j] )

Sharding: core = b*2 + g  (b in 0..3 data-parallel over batch,
g in 0..1 tensor-parallel over halves of C_OUT). No cross-core comm.

Per-core compute: host-side im2col gives patches T[p, r] with p = y*WO+x
(pixels, on partitions) and r = (c,i,j) (reduction, on free axis, R=200).
ONE fused DVE instruction per 128-pixel block computes all O_LOC=8 output
channels:
    accum[p, o] = max(init, max_r ( T[p, r] + w[o, r] ))
via a hand-authored custom DVE op (TTMR_SUBDIM):
  - in0 = T tile viewed [128, 8, 200] with a step-0 segment dim (re-read
    8x), in1 = the 8 weight rows broadcast across partitions [128, 1600];
  - runs in 2x_1port perf mode on fp16 streams (2 elems/lane/cycle);
  - a 5-state uop FSM (seed / steady / flush-read / flush-reseed /
    flush-final) max-accumulates in stage 3's CURR_ALU_OUT flop and, at
    each SUB_DIM_DONE segment boundary, writes the accumulator pair to
    the dst stream and reseeds — no READ_ACCUMULATOR instruction needed.
"""

import sys

import numpy as np

if "/opt/trn_rl_repo" not in sys.path:
    sys.path.insert(0, "/opt/trn_rl_repo")

B, C_IN, H, W = 4, 8, 128, 128
C_OUT, KH, KW = 16, 5, 5
HO, WO = H - KH + 1, W - KW + 1  # 124, 124
P = HO * WO  # 15376 output pixels per (b, o)
R = C_IN * KH * KW  # 200 reduction terms
NBLK = (P + 127) // 128  # 121 pixel blocks
PPAD = NBLK * 128  # 15488
OG = 2  # groups of output channels
O_LOC = C_OUT // OG  # 8 output channels per core = segments per instruction
N_CORES = 8

OP_NAME = "TTMR_SUBDIM"
ACC_INIT = -60000.0  # > -fp16_max; every real term beats it

# uop state ids
_SEED, _STEADY, _FRD, _FSEED, _FFIN = 0, 1, 2, 3, 4


def _build_uops():
    from concourse.dve_uop import (
        AluInp,
        AluOp,
        DelayInp,
        InpSel,
        OutPath,
        OutSel,
        Trigger,
        UopConfig,
        UopDpConfig,
    )

    inp = [
        InpSel.ZERO,
        InpSel.SRC_0,  # -> PREV_DELAY_0 at stage 0
        InpSel.SRC_1,  # -> PREV_DELAY_1
        InpSel.CONST_0,  # -> PREV_DELAY_2
        InpSel.SRC_0_HI,  # -> PREV_DELAY_3 (2x mode)
        InpSel.SRC_1_HI,  # -> PREV_DELAY_4 (2x mode)
        InpSel.ZERO,
        InpSel.ZERO,
    ]
    inp_en = [0, 1, 1, 1, 1, 1, 0, 0]

    def base(kind):
        u = UopConfig()
        u.inp = list(inp)
        u.inp_enable = list(inp_en)
        u.accum_enabled = 1
        if kind == _SEED:
            u.require_inp0 = 0
            u.require_inp1 = 0
            u.repeat_count = 1
            u.trigger = (Trigger.COUNT, Trigger.NONE, Trigger.NONE)
            u.next_uop = (_STEADY, 0, 0)
        elif kind == _STEADY:
            u.require_inp0 = 1
            u.require_inp1 = 1
            # priority: tensor-done (final flush) over segment boundary
            u.trigger = (Trigger.SRC_TENSOR_DONE, Trigger.SUB_DIM_DONE, Trigger.NONE)
            u.next_uop = (_FFIN, _FRD, 0)
        elif kind in (_FRD, _FFIN):
            u.require_inp0 = 0
            u.require_inp1 = 0
            u.repeat_count = 1
            u.trigger = (Trigger.COUNT, Trigger.NONE, Trigger.NONE)
            u.next_uop = (_FSEED if kind == _FRD else 0, 0, 0)
        else:  # _FSEED
            u.require_inp0 = 0
            u.require_inp1 = 0
            u.repeat_count = 1
            u.trigger = (Trigger.COUNT, Trigger.NONE, Trigger.NONE)
            u.next_uop = (_STEADY, 0, 0)
        return u

    def byp(a_inp=AluInp.PREV_ALU_OUT, lanes=(0, 1)):
        d = UopDpConfig().enable_alu(AluOp.BYPASS, a_inp)
        d.pass_through_delay(*lanes)
        return d

    def seed_dp_1x():
        d0 = UopDpConfig().enable_alu(
            AluOp.ADD, AluInp.PREV_DELAY_0, AluInp.PREV_DELAY_1
        )
        d0.pass_through_delay(0, 1, 2)
        d1 = byp(AluInp.PREV_DELAY_2, lanes=(0, 1, 2))  # CONST_0 -> stage1 flop
        return [d0, d1] + [byp() for _ in range(6)]

    def seed_dp_2x():
        d0 = UopDpConfig().enable_alu(
            AluOp.ADD, AluInp.PREV_DELAY_0, AluInp.PREV_DELAY_1
        )
        d0.pass_through_delay(0, 1, 2, 3, 4)
        d1 = byp(AluInp.PREV_DELAY_2, lanes=(1, 2, 3, 4))  # CONST_0 onto ALU path
        d1.enable_delay_from_src(DelayInp.PREV_ALU_OUT, 0)
        return [d0, d1, byp(), byp()] + [byp() for _ in range(4)]  # flop at stage 3

    def flush_dp_1x():
        d0 = UopDpConfig().enable_alu(AluOp.BYPASS, AluInp.PREV_DELAY_0)
        d1 = UopDpConfig().enable_alu(AluOp.BYPASS, AluInp.CURR_ALU_OUT)
        return [d0, d1] + [byp() for _ in range(6)]

    def flush_dp_2x():
        d0 = UopDpConfig().enable_alu(AluOp.BYPASS, AluInp.PREV_DELAY_0)
        d3 = UopDpConfig().enable_alu(AluOp.BYPASS, AluInp.CURR_ALU_OUT)
        d3.pass_through_delay(0, 1)
        return [d0, byp(), byp(), d3] + [byp() for _ in range(4)]

    from concourse.dve_uop import OutPath as OP, OutSel as OS

    def make_1x():
        seed = base(_SEED)
        seed.datapath_config = seed_dp_1x()

        steady = base(_STEADY)
        d0 = UopDpConfig().enable_alu(
            AluOp.ADD, AluInp.PREV_DELAY_0, AluInp.PREV_DELAY_1
        )
        d0.pass_through_delay(0, 1, 2)
        d1 = UopDpConfig().enable_alu(
            AluOp.MAX, AluInp.CURR_ALU_OUT, AluInp.PREV_ALU_OUT
        )
        d1.enable_delay_from_src(DelayInp.PREV_ALU_OUT, 0).pass_through_delay(1, 2)
        steady.datapath_config = [d0, d1] + [byp() for _ in range(6)]

        out_states = []
        for kind in (_FRD, _FFIN):
            f = base(kind)
            f.repeat_count = 2  # two 1x writes == one 2x pair: same dst layout
            f.datapath_config = flush_dp_1x()
            f.enable_output(OS.ALU_OUT, OP.WR0_LO)
            out_states.append(f)
        frd, ffin = out_states

        fseed = base(_FSEED)
        fseed.datapath_config = seed_dp_1x()
        return [seed, steady, frd, fseed, ffin]

    def make_2x():
        seed = base(_SEED)
        seed.datapath_config = seed_dp_2x()

        steady = base(_STEADY)
        d0 = UopDpConfig().enable_alu(
            AluOp.ADD, AluInp.PREV_DELAY_0, AluInp.PREV_DELAY_1
        )
        d0.pass_through_delay(0, 1, 2, 3, 4)
        d1 = UopDpConfig().enable_alu(
            AluOp.ADD, AluInp.PREV_DELAY_3, AluInp.PREV_DELAY_4
        )
        d1.enable_delay_from_src(DelayInp.PREV_ALU_OUT, 0).pass_through_delay(
            1, 2, 3, 4
        )
        d2 = UopDpConfig().enable_alu(
            AluOp.MAX, AluInp.PREV_ALU_OUT, AluInp.PREV_DELAY_0
        )
        d2.enable_delay_from_src(DelayInp.PREV_ALU_OUT, 1).pass_through_delay(0)
        d3 = UopDpConfig().enable_alu(
            AluOp.MAX, AluInp.CURR_ALU_OUT, AluInp.PREV_ALU_OUT
        )
        d3.pass_through_delay(0, 1)
        steady.datapath_config = [d0, d1, d2, d3] + [byp() for _ in range(4)]

        out_states = []
        for kind in (_FRD, _FFIN):
            f = base(kind)
            f.datapath_config = flush_dp_2x()
            f.enable_output(OS.ALU_OUT, OP.WR0_LO)
            f.enable_output(OS.ALU_OUT, OP.WR0_HI)
            out_states.append(f)
        frd, ffin = out_states

        fseed = base(_FSEED)
        fseed.datapath_config = seed_dp_2x()
        return [seed, steady, frd, fseed, ffin]

    return make_1x(), make_2x()


_COMPILED: dict = {}


def _compile_spec(ver):
    if ver not in _COMPILED:
        import concourse.dve_ops as DO
        from concourse.dve_uop import DveOpSpec

        row = DO._SUB_OPCODE_FOR_NAME[OP_NAME]
        uops_1x, uops_2x = _build_uops()
        s = DveOpSpec(
            name=OP_NAME,
            opcode=row,
            uops=uops_1x,
            rd1_en=True,
            uops_2x=uops_2x,
            perf_max=1,
        )
        s.validate(ver)
        _COMPILED[ver] = s
    return _COMPILED[ver]


def _register_op():
    import concourse.dve_ops as DO
    from concourse.dve_spec import C0, Spec, Src0, Src1, maxx

    for op in DO.OPS:
        if op.name == OP_NAME:
            return op
    spec = Spec(body=Src0 + Src1, accum=maxx, accum_init=C0)
    row = max(DO._SUB_OPCODE_FOR_NAME.values()) + 1
    assert row < 0x20, "custom-DVE row field overflow"
    DO._SUB_OPCODE_FOR_NAME[OP_NAME] = row
    shas = {ver: _compile_spec(ver).sha(ver) for ver in ("v3", "v4")}

    class DveOp2x(DO.DveOp):
        def compile(self, ver):
            return _compile_spec(ver)

    op = DveOp2x(OP_NAME, spec, subdim=True, uops_sha=shas)
    DO.OPS.append(op)
    DO.CUSTOM_DVE_SPECS[OP_NAME] = spec
    return op


def _emit(nc, op, *, out, in0, in1, s0):
    """One fused max-plus-reduce over O_LOC segments (2x fp16 perf mode)."""
    import concourse.bass_isa as bass_isa
    import concourse.mybir as mybir
    from concourse.dve_ops import get_dve_sub_opcode
    from concourse.dve_table_gen import dve_ver_for

    vec = nc.vector
    if op.name not in nc.m.ant_custom_dve_ops:
        nc.m.ant_custom_dve_ops = sorted({*nc.m.ant_custom_dve_ops, op.name})
    op.compile(dve_ver_for(nc.trn_type))
    shape = bass_isa.CustomDveShape.TTSS
    isa_opcode = nc.isa.Opcode[
        f"NEURON_ISA_TPB_OPCODE_CUSTOM_DVE_ANT_{shape.slot()}"
    ].value
    ins = [
        vec.lower_ap(in0, for_isa=True, opt=False),
        vec.lower_ap(in1, for_isa=True, opt=False),
        mybir.ImmediateValue(dtype=mybir.dt.float32, value=float(s0)),
        mybir.ImmediateValue(dtype=mybir.dt.float32, value=0.0),
    ]
    outs = [vec.lower_ap(out, for_isa=True, opt=False)]
    return vec.add_instruction(
        bass_isa.InstCustomDveAnt(
            name=nc.get_next_instruction_name(),
            op_name=op.name,
            rd1_en=True,
            subdim=0x02,
            imm2=0.0,
            shape=shape,
            row=get_dve_sub_opcode(op.name),
            isa_opcode=isa_opcode,
            ins=ins,
            outs=outs,
            perf_max=1,
        )
    )


def _build_program():
    import concourse.bacc as bacc
    import concourse.mybir as mybir
    from concourse.tile import TileContext

    ttmr = _register_op()
    f16 = mybir.dt.float16
    nc = bacc.Bacc("TRN2", target_bir_lowering=False, debug=False)

    t_dram = nc.dram_tensor("t", [NBLK, 128, R], f16, kind="ExternalInput")
    wb_dram = nc.dram_tensor("wb", [128, O_LOC * R], f16, kind="ExternalInput")
    out_dram = nc.dram_tensor(
        "out", [NBLK, 128, 2 * O_LOC], f16, kind="ExternalOutput"
    )

    with TileContext(nc) as tc:
        with (
            tc.tile_pool(name="wbp", bufs=1) as wbp,
            tc.tile_pool(name="tin", bufs=4) as tinp,
            tc.tile_pool(name="op", bufs=4) as outp,
        ):
            wb = wbp.tile([128, O_LOC * R], f16)
            nc.sync.dma_start(out=wb[:, :], in_=wb_dram[:, :])
            for blk in range(NBLK):
                tin = tinp.tile([128, R], f16)
                nc.sync.dma_start(out=tin[:, :], in_=t_dram[blk, :, :])
                ot = outp.tile([128, 2 * O_LOC], f16)
                in0 = tin[:, :].unsqueeze(1).broadcast_to((128, O_LOC, R))
                _emit(
                    nc,
                    ttmr,
                    out=ot[:, :],
                    in0=in0,
                    in1=wb[:, :],
                    s0=ACC_INIT,
                )
                nc.sync.dma_start(out=out_dram[blk, :, :], in_=ot[:, :])
    nc.finalize()
    return nc


def _host_shards(img: np.ndarray, kern: np.ndarray):
    """im2col on host (fp16): per-batch patches + per-group broadcast weights."""
    from numpy.lib.stride_tricks import sliding_window_view

    kflip = kern[:, :, ::-1, ::-1]
    wmat = np.ascontiguousarray(kflip.reshape(C_OUT, R))  # [16, 200], r=(c,i,j)

    sw = sliding_window_view(img, (KH, KW), axis=(2, 3))  # [B,C,HO,WO,KH,KW]
    t_full = sw.transpose(0, 2, 3, 1, 4, 5).reshape(B, P, R)
    t_pad = np.zeros((B, PPAD, R), np.float16)
    t_pad[:, :P] = t_full.astype(np.float16)

    in_maps = []
    for core in range(N_CORES):
        b, g = divmod(core, OG)
        wb = np.tile(
            wmat[g * O_LOC : (g + 1) * O_LOC].reshape(1, O_LOC * R), (128, 1)
        ).astype(np.float16)
        in_maps.append(
            {
                "t": t_pad[b].reshape(NBLK, 128, R),
                "wb": wb,
            }
        )
    return in_maps


def _run(in_maps, trace=False, **kwargs):
    from concourse.bass_utils import run_bass_kernel_spmd

    nc = _build_program()
    return run_bass_kernel_spmd(
        nc, in_maps, core_ids=list(range(N_CORES)), trace=trace, **kwargs
    )


def kernel(**inputs) -> np.ndarray:
    img = np.ascontiguousarray(np.asarray(inputs["img"], dtype=np.float32))
    kern = np.ascontiguousarray(np.asarray(inputs["kernel"], dtype=np.float32))

    in_maps = _host_shards(img, kern)
    res = _run(in_maps)

    out = np.empty((B, C_OUT, HO, WO), np.float32)
    for core in range(N_CORES):
        b, g = divmod(core, OG)
        o_core = (
            res.results[core]["out"]
            .reshape(PPAD, 2 * O_LOC)[:P, ::2]
            .astype(np.float32)
        )  # [15376, 8]
        out[b, g * O_LOC : (g + 1) * O_LOC] = np.ascontiguousarray(o_core.T).reshape(
            O_LOC, HO, WO
        )
    return out
